# revision 4
# baseline (speedup 1.0000x reference)
"""ChebConv (K=6) message-passing kernel for 8 Trainium2 NeuronCores.

Math: the reference's GraphNetwork pass multiplies each node's features by a
per-node scalar s = (deg - in_w) / max(deg) (deg = segment_sum(edges, senders),
in_w = segment_sum(edges, receivers)), and the Chebyshev recurrence
Tx_k = 2*Tx_{k-1} - Tx_{k-2} stays rank-1 per node: Tx_k = (1 + k*(s-1)) * x.
Hence
    out = X @ WA + s * (X @ WB) + b_tot
with WA = sum_k (1-k) Wk[k], WB = sum_k k Wk[k], b_tot = sum_k bk[k] + bias.

Sharding: nodes block-sharded over 8 cores (12500 each, padded to 12544).
Edges are routed on the host (index permutation + zero fill only, no float
arithmetic) into per-node slot layouts so each segment-sum becomes a dense
on-device reduction.

Two launches (an in-kernel AllReduce costs far more than a second NEFF):
  A: deg row-reductions (sender-routed fp8 slot matrix) + per-partition
     max(deg); host takes the max of the 8x128 partial maxima (selection).
  B: everything else.  Edges arrive as an fp8 pack [slot 0..127, node] with
     send slots in partitions 0..63 and recv slots in 64..127; one PE matmul
     with a +-1 "signs" stationary yields u = deg - in_w broadcast across all
     128 partitions, exactly the layout the s-scaling multiply needs (no
     transpose, no DRAM round-trip, no DMA broadcast).  1/m is folded into
     WB.  out^T = WA^T X^T + WB'^T (u (*) X)^T + b, all-bf16 on the PE, PSUM
     evacuation on the scalar engine, DVE only does the u (*) X products.
"""

import sys

sys.path.insert(0, "/opt/trn_rl_repo")

import numpy as np
import ml_dtypes

import concourse.bacc as bacc
import concourse.bass as bass
import concourse.mybir as mybir
import concourse.tile as tile
from concourse.bass_utils import run_bass_kernel_spmd

N_NODES = 100000
F = 128
KCH = 6
NCORES = 8
NPC = N_NODES // NCORES       # 12500 nodes per core
T = (NPC + 127) // 128        # 98 node tiles per core
NPAD = T * 128                # 12544 (cols 12500.. are zero padding)
DS = 64                       # per-node send slots (>= max send degree)
DR = 64                       # per-node recv slots (>= max recv degree)

f32 = mybir.dt.float32
f16 = mybir.dt.float16
bf16 = mybir.dt.bfloat16
f8 = mybir.dt.float8e4
np_f8 = ml_dtypes.float8_e4m3
np_bf16 = ml_dtypes.bfloat16

TRACE = False
LAST = {}

_prog_cache = {}


def _build_edge_program():
    """Launch A: deg = rowsum(pse), dmax[p] = max_t deg[p, t]."""
    nc = bacc.Bacc("TRN2", target_bir_lowering=False, debug=False,
                   num_devices=NCORES)
    A = mybir.AluOpType
    X = mybir.AxisListType.X

    pse_d = nc.dram_tensor("pse", [128, T * DS], f8, kind="ExternalInput")
    pmax_d = nc.dram_tensor("pmax", [128, 1], f32, kind="ExternalOutput")

    CH = [(0, 49), (49, 49)]
    with tile.TileContext(nc) as tc:
        with (
            tc.tile_pool(name="edge", bufs=1) as edgep,
            tc.tile_pool(name="small", bufs=1) as smallp,
        ):
            deg = smallp.tile([128, T], f32)
            pse_sb = edgep.tile([128, T, DS], f8)
            for t0, n in CH:
                nc.sync.dma_start(
                    pse_sb[:, t0 : t0 + n, :],
                    pse_d[:, t0 * DS : (t0 + n) * DS].rearrange("p (t d) -> p t d", d=DS))
                nc.vector.tensor_reduce(deg[:, t0 : t0 + n], pse_sb[:, t0 : t0 + n, :],
                                        axis=X, op=A.add)
            dmax = smallp.tile([128, 1], f32)
            nc.vector.tensor_reduce(dmax[:, :], deg[:, :], axis=X, op=A.max)
            nc.sync.dma_start(pmax_d[:, :], dmax[:, :])

    nc.compile()
    return nc


def _build_main_program():
    """Launch B: out^T = WA^T X^T + (1/m) WB^T (u (*) X)^T + b_tot."""
    nc = bacc.Bacc("TRN2", target_bir_lowering=False, debug=False,
                   num_devices=NCORES)
    A = mybir.AluOpType
    X = mybir.AxisListType.X

    xt_d = nc.dram_tensor("xt", [F, NPAD], bf16, kind="ExternalInput")
    pk_d = nc.dram_tensor("pk", [128, NPAD], f8, kind="ExternalInput")
    wk_d = nc.dram_tensor("wk", [KCH, F, F], bf16, kind="ExternalInput")
    bkb_d = nc.dram_tensor("bkb", [1, (KCH + 1) * F], f32, kind="ExternalInput")
    mmax_d = nc.dram_tensor("mmax", [1, 1], f32, kind="ExternalInput")
    out_d = nc.dram_tensor("out", [F, NPAD], bf16, kind="ExternalOutput")

    XCH = 7                    # xt / pk / out DMA chunks (1792 cols each)
    CW = NPAD // XCH           # 1792
    GW = 448                   # matmul group width (PSUM bank = 512 f32 max)
    GPC = CW // GW             # 4 groups per chunk

    with tile.TileContext(nc) as tc:
        with (
            tc.tile_pool(name="const", bufs=1) as constp,
            tc.tile_pool(name="xt", bufs=1) as xtp,
            tc.tile_pool(name="pk", bufs=1) as pkp,
            tc.tile_pool(name="outp", bufs=1) as outp,
            tc.tile_pool(name="small", bufs=1) as smallp,
            tc.tile_pool(name="sx", bufs=6) as sxp,
            tc.tile_pool(name="psu", bufs=4, space="PSUM") as psu,
            tc.tile_pool(name="psf", bufs=4, space="PSUM") as psf,
        ):
            # ---- critical-path DMAs: xt chunk 0 first, then pack, weights --
            xt_sb, pk_sb = [], []
            xt_c = xtp.tile([128, CW], bf16, name="xt0")
            pk_c = pkp.tile([128, CW], f8, name="pk0")
            with tc.high_priority():
                m_bc = smallp.tile([128, 1], f32)
                map_ = mmax_d[0:1, 0:1]
                nc.sync.dma_start(m_bc[:, :], bass.AP(map_.tensor, map_.offset, [[0, 128], [1, 1]]))
                H2 = CW // 2
                nc.sync.dma_start(xt_c[:, :H2], xt_d[:, :H2])
                nc.sync.dma_start(xt_c[:, H2:], xt_d[:, H2:CW])
                nc.sync.dma_start(pk_c[:, :], pk_d[:, :CW])
                wk_sb = constp.tile([128, KCH, F], bf16)
                nc.sync.dma_start(wk_sb[:, :, :], wk_d.ap().rearrange("k p f -> p k f"))
                bkb_sb = constp.tile([1, (KCH + 1) * F], f32)
                nc.sync.dma_start(bkb_sb[:, :], bkb_d[:, :])
            xt_sb.append(xt_c)
            pk_sb.append(pk_c)

            # ---- weights: WA = W0 - W2 - 2W3 - 3W4 - 4W5,
            #               WB = (W1 + 2W2 + 3W3 + 4W4 + 5W5) / m ------------
            with tc.high_priority():
                wab = constp.tile([128, 2 * F], f32)
                wa, wb = wab[:, 0:F], wab[:, F : 2 * F]
                nc.vector.scalar_tensor_tensor(wa, wk_sb[:, 2, :], -1.0, wk_sb[:, 0, :], op0=A.mult, op1=A.add)
                nc.vector.scalar_tensor_tensor(wa, wk_sb[:, 3, :], -2.0, wa, op0=A.mult, op1=A.add)
                nc.vector.scalar_tensor_tensor(wa, wk_sb[:, 4, :], -3.0, wa, op0=A.mult, op1=A.add)
                nc.vector.scalar_tensor_tensor(wa, wk_sb[:, 5, :], -4.0, wa, op0=A.mult, op1=A.add)
                nc.vector.scalar_tensor_tensor(wb, wk_sb[:, 2, :], 2.0, wk_sb[:, 1, :], op0=A.mult, op1=A.add)
                nc.vector.scalar_tensor_tensor(wb, wk_sb[:, 3, :], 3.0, wb, op0=A.mult, op1=A.add)
                nc.vector.scalar_tensor_tensor(wb, wk_sb[:, 4, :], 4.0, wb, op0=A.mult, op1=A.add)
                nc.vector.scalar_tensor_tensor(wb, wk_sb[:, 5, :], 5.0, wb, op0=A.mult, op1=A.add)
                minv = smallp.tile([128, 1], f32)
                nc.vector.reciprocal(minv[:, :], m_bc[:, :])
                wa16 = constp.tile([128, F], bf16)
                wb16 = constp.tile([128, F], bf16)
                nc.vector.tensor_copy(wa16[:, :], wa)
                nc.vector.tensor_scalar_mul(wb16[:, :], wb, minv[:, 0:1])

                # signs stationary: +1 on send slots (p<64), -1 on recv slots
                signs = constp.tile([128, 128], f8)
                nc.vector.memset(signs[0:64, :], 1.0)
                nc.vector.memset(signs[64:128, :], -1.0)

            # ---- bias column + ACT table pre-warm --------------------------
            with tc.high_priority():
                act_warm = smallp.tile([1, 1], f32)
                nc.scalar.activation(act_warm[:, :], m_bc[0:1, 0:1],
                                     mybir.ActivationFunctionType.Identity,
                                     bias=0.0, scale=1.0)
                btot = smallp.tile([1, F], f32)
                nc.vector.tensor_reduce(
                    btot[:, :], bkb_sb.rearrange("p (s f) -> p f s", s=KCH + 1),
                    axis=X, op=A.add)
                one1 = smallp.tile([1, 1], f32)
                nc.vector.memset(one1[:, :], 1.0)
                ps_bc = psf.tile([128, 1], f32, tag="psf")
                nc.tensor.matmul(ps_bc[:, :], btot[:, :], one1[:, :],
                                 start=True, stop=True)
                btot_col = smallp.tile([128, 1], f32)
                nc.vector.tensor_copy(btot_col[:, :], ps_bc[:, :])

            # ---- remaining bulk DMAs --------------------------------------
            for c in range(1, XCH):
                xt_c = xtp.tile([128, CW], bf16, name=f"xt{c}")
                pk_c = pkp.tile([128, CW], f8, name=f"pk{c}")
                nc.sync.dma_start(pk_c[:, :], pk_d[:, c * CW : (c + 1) * CW])
                nc.sync.dma_start(xt_c[:, :], xt_d[:, c * CW : (c + 1) * CW])
                xt_sb.append(xt_c)
                pk_sb.append(pk_c)

            # ---- main loop: chunk == quad of 4 groups ----------------------
            # PE order per quad: wa x4 (no sx dep), u x4, wb x4 (sx computed
            # by DVE while PE runs the next quad's wa/u block).
            for c in range(XCH):
                out_c = outp.tile([128, CW], bf16, name=f"out{c}")
                n0s = [g * GW for g in range(GPC)]
                psFs = []
                for n0 in n0s:
                    psF = psf.tile([128, GW], f32, tag="psf")
                    nc.tensor.matmul(psF[:, :], wa16[:, :],
                                     xt_sb[c][:, n0 : n0 + GW], start=True, stop=False)
                    psFs.append(psF)
                us = []
                for n0 in n0s:
                    u_ps = psu.tile([128, GW], f32, tag="psu")
                    nc.tensor.matmul(u_ps[:, :], signs[:, :],
                                     pk_sb[c][:, n0 : n0 + GW], start=True, stop=True)
                    us.append(u_ps)
                sxs = []
                for n0, u_ps in zip(n0s, us):
                    sx = sxp.tile([128, GW], bf16, tag="sx")
                    nc.vector.tensor_tensor(sx[:, :], xt_sb[c][:, n0 : n0 + GW],
                                            u_ps[:, :], op=A.mult)
                    sxs.append(sx)
                for sx, psF in zip(sxs, psFs):
                    nc.tensor.matmul(psF[:, :], wb16[:, :], sx[:, :],
                                     start=False, stop=True)
                # evacuate PSUM + bias on the scalar engine; DMA out per pair
                for gi, (n0, psF) in enumerate(zip(n0s, psFs)):
                    nc.scalar.activation(out_c[:, n0 : n0 + GW], psF[:, :],
                                         mybir.ActivationFunctionType.Identity,
                                         bias=btot_col[:, 0:1], scale=1.0)
                    if gi == 1:
                        nc.sync.dma_start(out_d[:, c * CW : c * CW + 2 * GW],
                                          out_c[:, : 2 * GW])
                    elif gi == 3:
                        nc.sync.dma_start(out_d[:, c * CW + 2 * GW : (c + 1) * CW],
                                          out_c[:, 2 * GW :])

    nc.compile()
    return nc


def _route_pse(vals8, idx):
    """Sender-routed slot matrix [128, T*DS] fp8 per core (launch A).
    Node ln of a core sits at row ln%128, tile ln//128; its edges occupy
    slots 0..deg-1 of that tile row.  Permutation + zero fill only."""
    order = np.argsort(idx, kind="stable")
    si = idx[order]
    sv = vals8[order]
    cnt = np.bincount(idx, minlength=N_NODES)
    assert cnt.max() <= DS, f"send degree {cnt.max()} > {DS}"
    first = np.concatenate(([0], np.cumsum(cnt)[:-1]))
    slot = np.arange(idx.shape[0], dtype=np.int64) - first[si]
    core = si // NPC
    ln = si - core * NPC
    rows = ln % 128
    cols = (ln // 128) * DS + slot
    packed = np.zeros((NCORES, 128, T * DS), np_f8)
    packed[core, rows, cols] = sv
    return packed


def _route_pack(vals8, senders, receivers):
    """fp8 pack [128, NPAD] per core: column = local node id, partitions
    0..DS-1 = that node's outgoing edge weights (sender-routed), partitions
    DS..127 = incoming edge weights (receiver-routed).  Permutation + zero
    fill only."""
    packed = np.zeros((NCORES, 128, NPAD), np_f8)
    for base, idx, ns in ((0, senders, DS), (DS, receivers, DR)):
        order = np.argsort(idx, kind="stable")
        si = idx[order]
        sv = vals8[order]
        cnt = np.bincount(idx, minlength=N_NODES)
        assert cnt.max() <= ns, f"degree {cnt.max()} > {ns}"
        first = np.concatenate(([0], np.cumsum(cnt)[:-1]))
        slot = np.arange(idx.shape[0], dtype=np.int64) - first[si]
        core = si // NPC
        ln = si - core * NPC
        packed[core, base + slot, ln] = sv
    return packed


def kernel(nodes, edges, senders, receivers, Wk, bk, bias):
    nodes = np.asarray(nodes, np.float32)
    edges = np.asarray(edges, np.float32)
    senders = np.asarray(senders)
    receivers = np.asarray(receivers)
    Wk = np.ascontiguousarray(np.asarray(Wk).astype(np_bf16))
    bk = np.asarray(bk, np.float32)
    bias = np.asarray(bias, np.float32)
    assert nodes.shape == (N_NODES, F) and Wk.shape == (KCH, F, F)

    if "edge" not in _prog_cache:
        _prog_cache["edge"] = _build_edge_program()
    if "main" not in _prog_cache:
        _prog_cache["main"] = _build_main_program()
    ncA = _prog_cache["edge"]
    ncB = _prog_cache["main"]

    v8 = edges.astype(np_f8)
    pse = _route_pse(v8, senders)
    pack = _route_pack(v8, senders, receivers)
    bkb = np.ascontiguousarray(
        np.concatenate([bk.reshape(1, -1), bias.reshape(1, -1)], axis=1), np.float32)

    cores = list(range(NCORES))
    in_a = [{"pse": np.ascontiguousarray(pse[c])} for c in cores]
    res_a = run_bass_kernel_spmd(ncA, in_a, cores, trace=TRACE)

    # combine the 8x128 device-computed partial maxima (selection only)
    m = max(float(res_a.results[c]["pmax"].max()) for c in cores)
    mmax = np.array([[m]], np.float32)

    in_b = []
    for c in cores:
        xt = np.zeros((F, NPAD), np_bf16)
        xt[:, :NPC] = nodes[c * NPC : (c + 1) * NPC].T
        in_b.append({
            "xt": xt,
            "pk": np.ascontiguousarray(pack[c]),
            "wk": Wk,
            "bkb": bkb,
            "mmax": mmax,
        })
    res_b = run_bass_kernel_spmd(ncB, in_b, cores, trace=TRACE)

    ta = res_a.exec_time_ns
    tb = res_b.exec_time_ns
    LAST["exec_a_ns"] = ta
    LAST["exec_b_ns"] = tb
    LAST["exec_time_ns"] = (ta + tb) if (ta is not None and tb is not None) else None

    out = np.empty((N_NODES, F), np.float32)
    for c in cores:
        o = res_b.results[c]["out"]
        out[c * NPC : (c + 1) * NPC] = np.asarray(o).astype(np.float32).T[:NPC]
    return out


# revision 6
# speedup vs baseline: 1.1034x; 1.1034x over previous
"""ChebConv (K=6) message-passing kernel for 8 Trainium2 NeuronCores.

Math: the reference's GraphNetwork pass multiplies each node's features by a
per-node scalar s = (deg - in_w) / max(deg) (deg = segment_sum(edges, senders),
in_w = segment_sum(edges, receivers)), and the Chebyshev recurrence
Tx_k = 2*Tx_{k-1} - Tx_{k-2} stays rank-1 per node: Tx_k = (1 + k*(s-1)) * x.
Hence
    out = X @ WA + s * (X @ WB) + b_tot
with WA = sum_k (1-k) Wk[k], WB = sum_k k Wk[k], b_tot = sum_k bk[k] + bias.

Sharding: nodes block-sharded over 8 cores (12500 each, padded to 12544).
Edges are routed on the host (index permutation + zero fill only, no float
arithmetic) into per-node slot layouts so each segment-sum becomes a dense
on-device reduction.

Two launches (an in-kernel AllReduce costs far more than a second NEFF):
  A: deg row-reductions (sender-routed fp8 slot matrix, split DVE/GpSimd) +
     per-core max(deg); host maxes the 8 partial maxima (selection only).
  B: everything else.  Edges arrive as an fp8 pack [slot 0..127, node] with
     send slots in partitions 0..63 and recv slots in 64..127; one PE matmul
     with a +-1 "signs" stationary yields u = deg - in_w broadcast across all
     128 partitions, exactly the layout the s-scaling multiply needs (no
     transpose, no DRAM round-trip, no DMA broadcast).  1/m is folded into
     WB.  out^T = WA^T X^T + WB'^T (u (*) X)^T + b, all-bf16 on the PE, PSUM
     evacuation on the scalar engine, DVE only does the u (*) X products.
"""

import sys

sys.path.insert(0, "/opt/trn_rl_repo")

import numpy as np
import ml_dtypes

import concourse.bacc as bacc
import concourse.bass as bass
import concourse.mybir as mybir
import concourse.tile as tile
from concourse import bass_isa
from concourse.bass_utils import run_bass_kernel_spmd

N_NODES = 100000
F = 128
KCH = 6
NCORES = 8
NPC = N_NODES // NCORES       # 12500 nodes per core
T = (NPC + 127) // 128        # 98 node tiles per core
NPAD = T * 128                # 12544 (cols 12500.. are zero padding)
DS = 64                       # per-node send slots (>= max send degree)
DR = 64                       # per-node recv slots (>= max recv degree)

f32 = mybir.dt.float32
bf16 = mybir.dt.bfloat16
f8 = mybir.dt.float8e4
np_f8 = ml_dtypes.float8_e4m3
np_bf16 = ml_dtypes.bfloat16

TRACE = False
LAST = {}

_prog_cache = {}


def _build_edge_program():
    """Launch A: deg = rowsum(pse), pmax = max(deg) (partition_all_reduce)."""
    nc = bacc.Bacc("TRN2", target_bir_lowering=False, debug=False,
                   num_devices=NCORES)
    A = mybir.AluOpType
    X = mybir.AxisListType.X

    pse_d = nc.dram_tensor("pse", [128, T * DS], f8, kind="ExternalInput")
    pmax_d = nc.dram_tensor("pmax", [1, 1], f32, kind="ExternalOutput")

    CH = [(0, 25, "v"), (25, 25, "v"), (50, 24, "v"), (74, 24, "v")]
    with tile.TileContext(nc) as tc:
        with (
            tc.tile_pool(name="edge", bufs=1) as edgep,
            tc.tile_pool(name="small", bufs=1) as smallp,
        ):
            deg = smallp.tile([128, T], f32)
            pse_sb = edgep.tile([128, T, DS], f8)
            for t0, n, eng in CH:
                nc.sync.dma_start(
                    pse_sb[:, t0 : t0 + n, :],
                    pse_d[:, t0 * DS : (t0 + n) * DS].rearrange("p (t d) -> p t d", d=DS))
                e = nc.vector if eng == "v" else nc.gpsimd
                e.tensor_reduce(deg[:, t0 : t0 + n], pse_sb[:, t0 : t0 + n, :],
                                axis=X, op=A.add)
            dmax = smallp.tile([128, 1], f32)
            nc.vector.tensor_reduce(dmax[:, :], deg[:, :], axis=X, op=A.max)
            pmax = smallp.tile([128, 1], f32)
            nc.gpsimd.partition_all_reduce(pmax[:, :], dmax[:, :], channels=128,
                                           reduce_op=bass_isa.ReduceOp.max)
            nc.sync.dma_start(pmax_d[:, :], pmax[0:1, 0:1])

    nc.compile()
    return nc


def _build_main_program():
    """Launch B: out^T = WA^T X^T + (1/m) WB^T (u (*) X)^T + b_tot."""
    nc = bacc.Bacc("TRN2", target_bir_lowering=False, debug=False,
                   num_devices=NCORES)
    A = mybir.AluOpType
    X = mybir.AxisListType.X

    xt_d = nc.dram_tensor("xt", [F, NPAD], bf16, kind="ExternalInput")
    pk_d = nc.dram_tensor("pk", [128, NPAD], f8, kind="ExternalInput")
    wk_d = nc.dram_tensor("wk", [KCH, F, F], bf16, kind="ExternalInput")
    bkb_d = nc.dram_tensor("bkb", [1, (KCH + 1) * F], f32, kind="ExternalInput")
    mmax_d = nc.dram_tensor("mmax", [1, 1], f32, kind="ExternalInput")
    out_d = nc.dram_tensor("out", [F, NPAD], bf16, kind="ExternalOutput")

    XCH = 7                    # xt / pk / out DMA chunks (1792 cols each)
    CW = NPAD // XCH           # 1792
    GW = 448                   # matmul group width (PSUM bank = 512 f32 max)
    GPC = CW // GW             # 4 groups per chunk

    with tile.TileContext(nc) as tc:
        with (
            tc.tile_pool(name="const", bufs=1) as constp,
            tc.tile_pool(name="xt", bufs=1) as xtp,
            tc.tile_pool(name="pk", bufs=1) as pkp,
            tc.tile_pool(name="outp", bufs=1) as outp,
            tc.tile_pool(name="small", bufs=1) as smallp,
            tc.tile_pool(name="sx", bufs=6) as sxp,
            tc.tile_pool(name="psu", bufs=4, space="PSUM") as psu,
            tc.tile_pool(name="psf", bufs=4, space="PSUM") as psf,
        ):
            # signs stationary (no input deps -> ready immediately):
            # +1 on send slots (p<64), -1 on recv slots
            signs = constp.tile([128, 128], f8)
            nc.vector.memset(signs[0:64, :], 1.0)
            nc.vector.memset(signs[64:128, :], -1.0)

            # ---- critical-path DMAs: wk first (feeds the serial weight
            # chain), then the first 448-col pieces of pk/xt ---------------
            xt_sb, pk_sb = [], []
            xt_c = xtp.tile([128, CW], bf16, name="xt0")
            pk_c = pkp.tile([128, CW], f8, name="pk0")
            with tc.high_priority():
                wk_sb = constp.tile([128, KCH, F], bf16)
                nc.sync.dma_start(wk_sb[:, :, :], wk_d.ap().rearrange("k p f -> p k f"))
                m_bc = smallp.tile([128, 1], f32)
                map_ = mmax_d[0:1, 0:1]
                nc.sync.dma_start(m_bc[:, :], bass.AP(map_.tensor, map_.offset, [[0, 128], [1, 1]]))
                for q in range(GPC):
                    nc.sync.dma_start(pk_c[:, q * GW : (q + 1) * GW],
                                      pk_d[:, q * GW : (q + 1) * GW])
                    nc.sync.dma_start(xt_c[:, q * GW : (q + 1) * GW],
                                      xt_d[:, q * GW : (q + 1) * GW])
                bkb_sb = constp.tile([1, (KCH + 1) * F], f32)
                nc.sync.dma_start(bkb_sb[:, :], bkb_d[:, :])
            xt_sb.append(xt_c)
            pk_sb.append(pk_c)

            # ---- weights: WA = W0 - W2 - 2W3 - 3W4 - 4W5  (chain emitted
            # first so wa16 is ready ASAP), WB = (W1 + ... + 5W5) / m -------
            with tc.high_priority():
                wab = constp.tile([128, 2 * F], f32)
                wa, wb = wab[:, 0:F], wab[:, F : 2 * F]
                nc.vector.scalar_tensor_tensor(wa, wk_sb[:, 2, :], -1.0, wk_sb[:, 0, :], op0=A.mult, op1=A.add)
                nc.vector.scalar_tensor_tensor(wa, wk_sb[:, 3, :], -2.0, wa, op0=A.mult, op1=A.add)
                nc.vector.scalar_tensor_tensor(wa, wk_sb[:, 4, :], -3.0, wa, op0=A.mult, op1=A.add)
                nc.vector.scalar_tensor_tensor(wa, wk_sb[:, 5, :], -4.0, wa, op0=A.mult, op1=A.add)
                wa16 = constp.tile([128, F], bf16)
                nc.vector.tensor_copy(wa16[:, :], wa)
                nc.vector.scalar_tensor_tensor(wb, wk_sb[:, 2, :], 2.0, wk_sb[:, 1, :], op0=A.mult, op1=A.add)
                nc.vector.scalar_tensor_tensor(wb, wk_sb[:, 3, :], 3.0, wb, op0=A.mult, op1=A.add)
                nc.vector.scalar_tensor_tensor(wb, wk_sb[:, 4, :], 4.0, wb, op0=A.mult, op1=A.add)
                nc.vector.scalar_tensor_tensor(wb, wk_sb[:, 5, :], 5.0, wb, op0=A.mult, op1=A.add)
                minv = smallp.tile([128, 1], f32)
                nc.vector.reciprocal(minv[:, :], m_bc[:, :])
                wb16 = constp.tile([128, F], bf16)
                nc.vector.tensor_scalar_mul(wb16[:, :], wb, minv[:, 0:1])

            # ---- bias column + ACT table pre-warm --------------------------
            with tc.high_priority():
                act_warm = smallp.tile([1, 1], f32)
                nc.scalar.activation(act_warm[:, :], m_bc[0:1, 0:1],
                                     mybir.ActivationFunctionType.Identity,
                                     bias=0.0, scale=1.0)
                btot = smallp.tile([1, F], f32)
                nc.vector.tensor_reduce(
                    btot[:, :], bkb_sb.rearrange("p (s f) -> p f s", s=KCH + 1),
                    axis=X, op=A.add)
                one1 = smallp.tile([1, 1], f32)
                nc.vector.memset(one1[:, :], 1.0)
                ps_bc = psf.tile([128, 1], f32, tag="psf")
                nc.tensor.matmul(ps_bc[:, :], btot[:, :], one1[:, :],
                                 start=True, stop=True)
                btot_col = smallp.tile([128, 1], f32)
                nc.vector.tensor_copy(btot_col[:, :], ps_bc[:, :])

            # ---- remaining bulk DMAs --------------------------------------
            for c in range(1, XCH):
                xt_c = xtp.tile([128, CW], bf16, name=f"xt{c}")
                pk_c = pkp.tile([128, CW], f8, name=f"pk{c}")
                nc.sync.dma_start(pk_c[:, :], pk_d[:, c * CW : (c + 1) * CW])
                nc.sync.dma_start(xt_c[:, :], xt_d[:, c * CW : (c + 1) * CW])
                xt_sb.append(xt_c)
                pk_sb.append(pk_c)

            # ---- main loop: chunk == quad of 4 groups ----------------------
            # PE order per quad: u x4 (needs only pk+signs), wa x4, wb x4;
            # DVE computes sx for quad q while PE runs u/wa of quad q+1.
            for c in range(XCH):
                out_c = outp.tile([128, CW], bf16, name=f"out{c}")
                n0s = [g * GW for g in range(GPC)]
                us = []
                for n0 in n0s:
                    u_ps = psu.tile([128, GW], f32, tag="psu")
                    nc.tensor.matmul(u_ps[:, :], signs[:, :],
                                     pk_sb[c][:, n0 : n0 + GW], start=True, stop=True)
                    us.append(u_ps)
                sxs = []
                for n0, u_ps in zip(n0s, us):
                    sx = sxp.tile([128, GW], bf16, tag="sx")
                    nc.vector.tensor_tensor(sx[:, :], xt_sb[c][:, n0 : n0 + GW],
                                            u_ps[:, :], op=A.mult)
                    sxs.append(sx)
                psFs = []
                for n0 in n0s:
                    psF = psf.tile([128, GW], f32, tag="psf")
                    nc.tensor.matmul(psF[:, :], wa16[:, :],
                                     xt_sb[c][:, n0 : n0 + GW], start=True, stop=False)
                    psFs.append(psF)
                for sx, psF in zip(sxs, psFs):
                    nc.tensor.matmul(psF[:, :], wb16[:, :], sx[:, :],
                                     start=False, stop=True)
                # evacuate PSUM + bias; last chunk alternates Scalar/DVE so
                # the tail drains twice as fast
                for gi, (n0, psF) in enumerate(zip(n0s, psFs)):
                    if c == XCH - 1 and gi in (1, 3):
                        nc.vector.tensor_scalar_add(out_c[:, n0 : n0 + GW],
                                                    psF[:, :], btot_col[:, 0:1])
                    else:
                        nc.scalar.activation(out_c[:, n0 : n0 + GW], psF[:, :],
                                             mybir.ActivationFunctionType.Identity,
                                             bias=btot_col[:, 0:1], scale=1.0)
                    if gi == 1:
                        nc.sync.dma_start(out_d[:, c * CW : c * CW + 2 * GW],
                                          out_c[:, : 2 * GW])
                    elif gi == 3:
                        nc.sync.dma_start(out_d[:, c * CW + 2 * GW : (c + 1) * CW],
                                          out_c[:, 2 * GW :])

    nc.compile()
    return nc


def _route_pse(vals8, idx):
    """Sender-routed slot matrix [128, T*DS] fp8 per core (launch A).
    Node ln of a core sits at row ln%128, tile ln//128; its edges occupy
    slots 0..deg-1 of that tile row.  Permutation + zero fill only."""
    order = np.argsort(idx, kind="stable")
    si = idx[order]
    sv = vals8[order]
    cnt = np.bincount(idx, minlength=N_NODES)
    assert cnt.max() <= DS, f"send degree {cnt.max()} > {DS}"
    first = np.concatenate(([0], np.cumsum(cnt)[:-1]))
    slot = np.arange(idx.shape[0], dtype=np.int64) - first[si]
    core = si // NPC
    ln = si - core * NPC
    rows = ln % 128
    cols = (ln // 128) * DS + slot
    packed = np.zeros((NCORES, 128, T * DS), np_f8)
    packed[core, rows, cols] = sv
    return packed


def _route_pack(vals8, senders, receivers):
    """fp8 pack [128, NPAD] per core: column = local node id, partitions
    0..DS-1 = that node's outgoing edge weights (sender-routed), partitions
    DS..127 = incoming edge weights (receiver-routed).  Permutation + zero
    fill only."""
    packed = np.zeros((NCORES, 128, NPAD), np_f8)
    for base, idx, ns in ((0, senders, DS), (DS, receivers, DR)):
        order = np.argsort(idx, kind="stable")
        si = idx[order]
        sv = vals8[order]
        cnt = np.bincount(idx, minlength=N_NODES)
        assert cnt.max() <= ns, f"degree {cnt.max()} > {ns}"
        first = np.concatenate(([0], np.cumsum(cnt)[:-1]))
        slot = np.arange(idx.shape[0], dtype=np.int64) - first[si]
        core = si // NPC
        ln = si - core * NPC
        packed[core, base + slot, ln] = sv
    return packed


def kernel(nodes, edges, senders, receivers, Wk, bk, bias):
    nodes = np.asarray(nodes, np.float32)
    edges = np.asarray(edges, np.float32)
    senders = np.asarray(senders)
    receivers = np.asarray(receivers)
    Wk = np.ascontiguousarray(np.asarray(Wk).astype(np_bf16))
    bk = np.asarray(bk, np.float32)
    bias = np.asarray(bias, np.float32)
    assert nodes.shape == (N_NODES, F) and Wk.shape == (KCH, F, F)

    if "edge" not in _prog_cache:
        _prog_cache["edge"] = _build_edge_program()
    if "main" not in _prog_cache:
        _prog_cache["main"] = _build_main_program()
    ncA = _prog_cache["edge"]
    ncB = _prog_cache["main"]

    v8 = edges.astype(np_f8)
    pse = _route_pse(v8, senders)
    pack = _route_pack(v8, senders, receivers)
    bkb = np.ascontiguousarray(
        np.concatenate([bk.reshape(1, -1), bias.reshape(1, -1)], axis=1), np.float32)

    cores = list(range(NCORES))
    in_a = [{"pse": np.ascontiguousarray(pse[c])} for c in cores]
    res_a = run_bass_kernel_spmd(ncA, in_a, cores, trace=TRACE)

    # combine the 8 device-computed partial maxima (selection only)
    m = max(float(res_a.results[c]["pmax"][0, 0]) for c in cores)
    mmax = np.array([[m]], np.float32)

    in_b = []
    for c in cores:
        xt = np.zeros((F, NPAD), np_bf16)
        xt[:, :NPC] = nodes[c * NPC : (c + 1) * NPC].T
        in_b.append({
            "xt": xt,
            "pk": np.ascontiguousarray(pack[c]),
            "wk": Wk,
            "bkb": bkb,
            "mmax": mmax,
        })
    res_b = run_bass_kernel_spmd(ncB, in_b, cores, trace=TRACE)

    ta = res_a.exec_time_ns
    tb = res_b.exec_time_ns
    LAST["exec_a_ns"] = ta
    LAST["exec_b_ns"] = tb
    LAST["exec_time_ns"] = (ta + tb) if (ta is not None and tb is not None) else None

    out = np.empty((N_NODES, F), np.float32)
    for c in cores:
        o = res_b.results[c]["out"]
        out[c * NPC : (c + 1) * NPC] = np.asarray(o).astype(np.float32).T[:NPC]
    return out
